# revision 1
# baseline (speedup 1.0000x reference)
"""Trainium2 Bass kernel: multi-head attention with sequence-axis layernorm
and relative position bias, sharded 8-way over heads (2 heads/core).

Layout strategy (all per core):
  - LN over sequence axis done in [d_partition, n_free] layout (xT input);
    g is folded into Wq/Wkv on the host.
  - qT/kT produced transposed [inner_local=128, b*n] (head-dim on partitions)
    so sim is computed TRANSPOSED: simT[nj, ni] = kT.T @ qT (K=dh=64), with
    the two local heads row-tiled into the PE array concurrently (rows 0-63 /
    64-127 via base_partition auto tile_position).
  - the bias add is folded multiplicatively: the host precomputes
    exp(biasT) (bf16); on-chip attn_u = exp(sim) * expb via a DVE bf16
    multiply (2x mode) in the otherwise-idle attention window.
  - softmax without max-subtraction (scores ~ N(0,2); exp safe in f32);
    ScalarE exp reads PSUM [128, 1024] spans directly, writes bf16 attn_uT.
  - av matmul: lhsT = v_aug [nj, 65] (ones column -> row 64 = Z), rhs =
    attn_uT, accumulated over nj into [65, 512] PSUM chunks.
  - normalization by 1/Z is folded into the OUTPUT projection: per-head
    PSUM partials scaled by per-partition 1/Z columns (Z round-trips
    through DRAM to transpose rows->columns).
"""

import numpy as np
import ml_dtypes

import concourse.bass as bass
from concourse import bacc
import concourse.mybir as mybir
import concourse.tile as tile
from concourse.masks import make_identity
from concourse.bass_utils import run_bass_kernel_spmd

F32 = mybir.dt.float32
BF16 = mybir.dt.bfloat16
BF = ml_dtypes.bfloat16
AF = mybir.ActivationFunctionType
ALU = mybir.AluOpType

# full-size problem constants
B, N, DIM = 2, 2048, 1024
HEADS, DH = 16, 64
NCORES = 8
HL = HEADS // NCORES          # heads per core = 2
IL = HL * DH                  # local inner = 128
INNER = HEADS * DH            # 1024


def build(b_sz=B, n_sz=N, dim=DIM, eps=1e-5):
    """Build the per-core Bass graph (SPMD across 8 cores)."""
    nd = dim // 128               # d tiles
    nch = (b_sz * n_sz) // 512    # 512-col chunks of flattened b*n
    njb = n_sz // 128             # key tiles per batch
    nic = n_sz // 512             # query chunks per batch
    bn = b_sz * n_sz
    nsub = n_sz // 512            # bn_stats subgroups

    nc = bacc.Bacc(None, target_bir_lowering=False)
    xT = nc.declare_dram_parameter("xT", [b_sz, dim, n_sz], BF16, isOutput=False)
    wqT = nc.declare_dram_parameter("wqT", [dim, IL], BF16, isOutput=False)
    wkT = nc.declare_dram_parameter("wkT", [dim, IL], BF16, isOutput=False)
    wvT = nc.declare_dram_parameter("wvT", [dim, IL], BF16, isOutput=False)
    woT = nc.declare_dram_parameter("woT", [IL, dim], BF16, isOutput=False)
    biasT = nc.declare_dram_parameter("biasT", [HL, n_sz, n_sz], BF16, isOutput=False)  # holds exp(bias.T)
    out = nc.declare_dram_parameter("out", [bn, dim], F32, isOutput=True)
    zdram = nc.dram_tensor("zscratch", [b_sz, HL, n_sz], BF16)
    zrdram = nc.dram_tensor("zrscratch", [b_sz, HL, 1, n_sz], BF16)

    with tile.TileContext(nc) as tc:
        with (
            tc.tile_pool(name="consts", bufs=1) as consts,
            tc.tile_pool(name="persist", bufs=1) as persist,
        ):
            # ---- load weights; build identity ----
            wq_s, wk_s, wv_s = [], [], []
            for dt in range(nd):
                for lst, src, nm in ((wq_s, wqT, "wq"), (wk_s, wkT, "wk"), (wv_s, wvT, "wv")):
                    t = consts.tile([128, IL], BF16, tag=f"{nm}{dt}")
                    nc.sync.dma_start(out=t, in_=src[dt * 128:(dt + 1) * 128, :])
                    lst.append(t)
            ident = consts.tile([128, 128], BF16, tag="ident")
            make_identity(nc, ident)
            wo_h = []
            for h in range(HL):
                t = consts.tile([DH, dim], BF16, tag=f"wo{h}")
                nc.sync.dma_start(out=t, in_=woT[h * DH:(h + 1) * DH, :])
                wo_h.append(t)

            xn = {}
            qT = persist.tile([IL, bn], BF16, tag="qT")
            kT = persist.tile([IL, bn], BF16, tag="kT")
            va = {}
            avz = {}   # (b, h) -> [DH+1, n] bf16, rows 0..63 = av_u, row 64 = Z

            # ---- Phase 1: layernorm over sequence axis ----
            with (
                tc.tile_pool(name="xload", bufs=3) as xload,
                tc.tile_pool(name="lns", bufs=8) as lns,
            ):
                for b in range(b_sz):
                    for dt in range(nd):
                        xt = xload.tile([128, n_sz], BF16, tag="xt")
                        nc.sync.dma_start(out=xt, in_=xT[b, dt * 128:(dt + 1) * 128, :])
                        stats = lns.tile([128, nsub, 6], F32, tag="stats")
                        for s in range(nsub):
                            nc.vector.bn_stats(out=stats[:, s, :], in_=xt[:, s * 512:(s + 1) * 512])
                        mv = lns.tile([128, 2], F32, tag="mv")
                        nc.vector.bn_aggr(out=mv, in_=stats)
                        vmax = lns.tile([128, 1], F32, tag="vmax")
                        nc.vector.tensor_scalar_max(vmax, mv[:, 1:2], eps)
                        sq = lns.tile([128, 1], F32, tag="sq")
                        nc.scalar.activation(out=sq, in_=vmax, func=AF.Sqrt)
                        scl = lns.tile([128, 1], F32, tag="scl")
                        nc.vector.reciprocal(scl, sq)
                        shf = lns.tile([128, 1], F32, tag="shf")
                        nc.vector.tensor_mul(shf, mv[:, 0:1], scl)
                        nshf = lns.tile([128, 1], F32, tag="nshf")
                        nc.vector.tensor_scalar_mul(nshf, shf, -1.0)
                        xnt = persist.tile([128, n_sz], BF16, tag=f"xn_{b}_{dt}")
                        nc.scalar.activation(out=xnt, in_=xt, func=AF.Identity,
                                             bias=nshf, scale=scl)
                        xn[b, dt] = xnt

            # ---- Phase 2a: q/k projections (transposed layout) ----
            with tc.tile_pool(name="pproj", bufs=4, space="PSUM") as pproj:
                for ch in range(nch):
                    b = (ch * 512) // n_sz
                    col0 = (ch * 512) % n_sz
                    for (w_s, dst) in ((wq_s, qT), (wk_s, kT)):
                        ps = pproj.tile([IL, 512], F32, tag="ps")
                        for dt in range(nd):
                            nc.tensor.matmul(
                                ps, w_s[dt], xn[b, dt][:, col0:col0 + 512],
                                start=(dt == 0), stop=(dt == nd - 1),
                            )
                        nc.scalar.activation(out=dst[:, ch * 512:(ch + 1) * 512],
                                             in_=ps, func=AF.Copy)

            # ---- Phase 2b: v natural + ones column ----
            with tc.tile_pool(name="pv", bufs=4, space="PSUM") as pv:
                for b in range(b_sz):
                    for nj in range(njb):
                        psv = pv.tile([128, IL], F32, tag="psv")
                        for dt in range(nd):
                            nc.tensor.matmul(
                                psv, xn[b, dt][:, nj * 128:(nj + 1) * 128], wv_s[dt],
                                start=(dt == 0), stop=(dt == nd - 1),
                            )
                        for h in range(HL):
                            t = persist.tile([128, DH + 1], BF16, tag=f"va_{b}_{h}_{nj}")
                            nc.vector.tensor_copy(t[:, 0:DH], psv[:, h * DH:(h + 1) * DH])
                            nc.vector.memset(t[:, DH:DH + 1], 1.0)
                            va[b, h, nj] = t

            # ---- Phase 3+4: attention with interleaved output projection ----
            for b in range(b_sz):
                for h in range(HL):
                    avz[b, h] = persist.tile([DH + 1, n_sz], BF16, tag=f"avz_{b}_{h}",
                                             name=f"avz_{b}_{h}")
            with (
                tc.tile_pool(name="psim", bufs=1, space="PSUM") as psim,
                tc.tile_pool(name="pav", bufs=1, space="PSUM") as pavp,
                
                tc.tile_pool(name="attnp", bufs=4) as attnp,
                tc.tile_pool(name="biasp", bufs=6) as biasp,
                tc.tile_pool(name="ost", bufs=4) as ost,
                tc.tile_pool(name="zc", bufs=2) as zc,
            ):
                # both batches interleaved per round: 4 independent streams
                # (b x h) hide the sim->exp->mult->av latency; bias tile shared
                # across batches (same head/nj/ni)
                for ni in range(nic):
                    pavs = {}
                    for b in range(b_sz):
                        for h in range(HL):
                            pavs[b, h] = pavp.tile(
                                [DH + 1, 512], F32, tag=f"pav{b}_{h}",
                                name=f"pav_{b}_{ni}_{h}")
                    for nj in range(njb):
                        pst = {}
                        for b in range(b_sz):
                            for h in range(HL):
                                pst[b, h] = psim.tile([128, 512], F32,
                                                      tag=f"ps{b}_{h}",
                                                      name=f"ps_{b}_{ni}_{h}_{nj}")
                                nc.tensor.matmul(
                                    pst[b, h],
                                    kT[h * DH:(h + 1) * DH,
                                       b * n_sz + nj * 128:b * n_sz + (nj + 1) * 128],
                                    qT[h * DH:(h + 1) * DH,
                                       b * n_sz + ni * 512:b * n_sz + (ni + 1) * 512],
                                    start=True, stop=True,
                                )
                        aus = {}
                        for h in range(HL):
                            bt = biasp.tile([128, 512], BF16, tag="bt", name="bt")
                            nc.sync.dma_start(
                                out=bt,
                                in_=biasT[h, nj * 128:(nj + 1) * 128,
                                          ni * 512:(ni + 1) * 512],
                            )
                            for b in range(b_sz):
                                ae = attnp.tile([128, 512], BF16,
                                                tag=f"ae{b}_{h}", name="ae")
                                nc.scalar.activation(out=ae, in_=pst[b, h], func=AF.Exp)
                                au = attnp.tile([128, 512], BF16,
                                                tag=f"au{b}_{h}", name="au")
                                nc.vector.tensor_mul(au, ae, bt)
                                aus[b, h] = au
                        for b in range(b_sz):
                            for h in range(HL):
                                nc.tensor.matmul(
                                    pavs[b, h], va[b, h, nj], aus[b, h],
                                    start=(nj == 0), stop=(nj == njb - 1),
                                )
                    for b in range(b_sz):
                        for h in range(HL):
                            nc.vector.tensor_copy(
                                avz[b, h][:, ni * 512:(ni + 1) * 512], pavs[b, h])
                            nc.sync.dma_start(
                                out=zdram[b, h, ni * 512:(ni + 1) * 512],
                                in_=avz[b, h][DH:DH + 1, ni * 512:(ni + 1) * 512])
                # ---- Z transpose roundtrip + normalize ----
                for b in range(b_sz):
                    zcol = zc.tile([128, HL, njb], BF16, tag="zcol", name="zcol")
                    nc.sync.dma_start(
                        out=zcol, in_=zdram[b].rearrange("h (c p) -> p h c", p=128))
                    zr = zc.tile([128, HL, njb], BF16, tag="zrb", name="zrb")
                    with nc.allow_low_precision(reason="1/Z bf16; ~4e-3 ok at 2e-2 gate"):
                        nc.vector.reciprocal(zr, zcol)
                    nc.sync.dma_start(
                        out=zrdram[b, :, 0, :].rearrange("h (c p) -> p h c", p=128),
                        in_=zr,
                    )
                    for h in range(HL):
                        zbb = zc.tile([DH, n_sz], BF16, tag="zbb", name="zbb")
                        nc.sync.dma_start(
                            out=zbb, in_=zrdram[b, h].to_broadcast([DH, n_sz]))
                        nc.vector.tensor_mul(avz[b, h][0:DH, :], avz[b, h][0:DH, :], zbb)
            with (
                tc.tile_pool(name="pout", bufs=2, space="PSUM") as pout,
                tc.tile_pool(name="ost2", bufs=3) as ost2,
            ):
                for blk in range(bn // 128):
                    b = (blk * 128) // n_sz
                    r0 = (blk * 128) % n_sz
                    po = pout.tile([128, dim], F32, tag="po", name="po")
                    for c0 in range(0, dim, 512):
                        w = min(512, dim - c0)
                        for h in range(HL):
                            nc.tensor.matmul(
                                po[:, c0:c0 + w],
                                avz[b, h][0:DH, r0:r0 + 128],
                                wo_h[h][:, c0:c0 + w],
                                start=(h == 0), stop=(h == HL - 1),
                            )
                    os_ = ost2.tile([128, dim], F32, tag="os", name="os")
                    nc.vector.tensor_copy(os_, po)
                    nc.sync.dma_start(out=out[blk * 128:(blk + 1) * 128, :], in_=os_)
    nc.compile()
    return nc


_NC_CACHE = {}


def _get_nc(key, **kw):
    if key not in _NC_CACHE:
        _NC_CACHE[key] = build(**kw)
    return _NC_CACHE[key]


def make_in_maps(x, rel_pos_bias, g, Wq, Wkv, Wo):
    b_sz, n_sz, dim = x.shape
    inner = Wq.shape[0]
    x = np.asarray(x, np.float32)
    xTh = np.ascontiguousarray(x.transpose(0, 2, 1)).astype(BF)  # [B, DIM, N]
    gv = np.asarray(g, np.float32).reshape(1, dim)
    Wq = np.asarray(Wq, np.float32) * gv
    Wkv = np.asarray(Wkv, np.float32) * gv
    scale = DH ** -0.5
    in_maps = []
    for c in range(NCORES):
        rs, re = c * IL, (c + 1) * IL
        wq_c = np.ascontiguousarray((Wq[rs:re, :] * scale).T).astype(BF)
        wk_c = np.ascontiguousarray(Wkv[rs:re, :].T).astype(BF)
        wv_c = np.ascontiguousarray(Wkv[inner + rs:inner + re, :].T).astype(BF)
        wo_c = np.ascontiguousarray(np.asarray(Wo)[:, rs:re].T).astype(BF)
        bias_c = np.exp(np.ascontiguousarray(
            np.asarray(rel_pos_bias)[0, c * HL:(c + 1) * HL].transpose(0, 2, 1)
        )).astype(BF)
        in_maps.append({
            "xT": xTh, "wqT": wq_c, "wkT": wk_c, "wvT": wv_c,
            "woT": wo_c, "biasT": bias_c,
        })
    return in_maps


def kernel(x, rel_pos_bias, g, Wq, Wkv, Wo):
    b_sz, n_sz, dim = x.shape
    nc = _get_nc((b_sz, n_sz, dim), b_sz=b_sz, n_sz=n_sz, dim=dim)
    in_maps = make_in_maps(x, rel_pos_bias, g, Wq, Wkv, Wo)
    res = run_bass_kernel_spmd(nc, in_maps, core_ids=list(range(NCORES)))
    acc = np.zeros((b_sz * n_sz, dim), np.float32)
    for r in res.results:
        acc += np.asarray(r["out"], np.float32)
    return np.ascontiguousarray(acc.reshape(b_sz, n_sz, dim))



# revision 7
# speedup vs baseline: 1.2107x; 1.2107x over previous
"""Trainium2 Bass kernel: multi-head attention with sequence-axis layernorm
and relative position bias, sharded 8-way over heads (2 heads/core).

v2 layout strategy (per core):
  - LN over sequence axis in [d_partition, n_free] layout; stats on DVE
    (bn_stats/bn_aggr), apply on DVE via fused tensor_scalar (x*scl + nshf)
    in bf16 (4x mode); g folded into Wq/Wkv on the host.
  - qT/kT [inner_local=128, b*n] via const-weight matmuls (K=128, Nf=512).
  - v natural per (b, nj): va_full[b,nj] [128 tokens, 128 inner] bf16.
  - attention rounds (ni, nj): all 4 streams (b x h) share ONE persistent
    4-bank PSUM tile [128, 2048] f32, cols [b0h0|b0h1|b1h0|b1h1]; the two
    sims of a batch are row-tiled (K=64 at row groups 0/64) into DIFFERENT
    banks and run concurrently on the PE.
  - exp: ONE ScalarE activation per batch-pair [128, 1024] spanning 2 PSUM
    banks (amortizes the ~352-cycle ACT instruction overhead).
  - bias folded multiplicatively: host precomputes exp(biasT) bf16; DVE
    multiplies (2x mode) into au.
  - AV: col-tiled pairs — h0 -> pav[b][0:64], h1 -> pav[b][64:128] (M=64,
    col groups disjoint -> concurrent), accumulated over nj. This yields a
    head-STACKED av [128, qi] enabling a K=128 output projection.
  - Z: separate [128,512] PSUM bank; 4 concurrent M=1 col-tiled matmuls
    with ones-weights at partitions {0,32,64,96}, accumulated over nj.
  - softmax denominator: reciprocal on DVE at ni boundary, DRAM roundtrip
    to broadcast 1/Z rows across 64 partitions; normalization deferred to
    the out-projection phase (off the attention critical path).
  - out-proj: stacked K=128 matmuls (lhsT = av_n [128, tok]), PSUM->SBUF
    copies alternating DVE/ACT, bf16 partial output summed on host in f32.
"""

import numpy as np
import ml_dtypes

import concourse.bass as bass
from concourse import bacc
import concourse.mybir as mybir
import concourse.tile as tile
from concourse.bass_utils import run_bass_kernel_spmd

F32 = mybir.dt.float32
BF16 = mybir.dt.bfloat16
BF = ml_dtypes.bfloat16
AF = mybir.ActivationFunctionType
ALU = mybir.AluOpType

# full-size problem constants
B, N, DIM = 2, 2048, 1024
HEADS, DH = 16, 64
NCORES = 8
HL = HEADS // NCORES          # heads per core = 2
IL = HL * DH                  # local inner = 128
INNER = HEADS * DH            # 1024


def build(b_sz=B, n_sz=N, dim=DIM, eps=1e-5):
    """Build the per-core Bass graph (SPMD across 8 cores)."""
    nd = dim // 128               # d tiles
    nch = (b_sz * n_sz) // 512    # 512-col chunks of flattened b*n
    njb = n_sz // 128             # key tiles per batch
    nic = n_sz // 512             # query chunks per batch
    bn = b_sz * n_sz
    nsub = n_sz // 512            # bn_stats subgroups

    nc = bacc.Bacc(None, target_bir_lowering=False)
    xT = nc.declare_dram_parameter("xT", [b_sz, dim, n_sz], BF16, isOutput=False)
    wqT = nc.declare_dram_parameter("wqT", [dim, IL], BF16, isOutput=False)
    wkT = nc.declare_dram_parameter("wkT", [dim, IL], BF16, isOutput=False)
    wvT = nc.declare_dram_parameter("wvT", [dim, IL], BF16, isOutput=False)
    woT = nc.declare_dram_parameter("woT", [IL, dim], BF16, isOutput=False)
    biasT = nc.declare_dram_parameter("biasT", [HL, n_sz, n_sz], BF16, isOutput=False)  # exp(bias.T)
    out = nc.declare_dram_parameter("out", [bn, dim], BF16, isOutput=True)
    zdram = nc.dram_tensor("zscratch", [b_sz, HL, 1, n_sz], BF16)

    with tile.TileContext(nc) as tc:
        with (
            tc.tile_pool(name="consts", bufs=1) as consts,
            tc.tile_pool(name="persist", bufs=1) as persist,
        ):
            # ---- load weights ----
            wq_s, wk_s, wv_s = [], [], []
            for dt in range(nd):
                for lst, src, nm in ((wq_s, wqT, "wq"), (wk_s, wkT, "wk"), (wv_s, wvT, "wv")):
                    t = consts.tile([128, IL], BF16, tag=f"{nm}{dt}")
                    nc.sync.dma_start(out=t, in_=src[dt * 128:(dt + 1) * 128, :])
                    lst.append(t)
            wo_full = consts.tile([IL, dim], BF16, tag="wo")
            nc.sync.dma_start(out=wo_full, in_=woT[:, :])
            ones = consts.tile([128, 1], BF16, tag="ones")
            nc.vector.memset(ones, 1.0)

            xn = {}
            qT = persist.tile([IL, bn], BF16, tag="qT")
            kT = persist.tile([IL, bn], BF16, tag="kT")
            va = {}   # (b, nj) -> [128 tokens, 128 inner] bf16

            # ---- Phase 1: layernorm over sequence axis (DVE only) ----
            with (
                tc.tile_pool(name="xload", bufs=3) as xload,
                tc.tile_pool(name="lns", bufs=8) as lns,
            ):
                for b in range(b_sz):
                    for dt in range(nd):
                        xt = xload.tile([128, n_sz], BF16, tag="xt")
                        nc.sync.dma_start(out=xt, in_=xT[b, dt * 128:(dt + 1) * 128, :])
                        stats = lns.tile([128, nsub, 6], F32, tag="stats")
                        for s in range(nsub):
                            nc.vector.bn_stats(out=stats[:, s, :], in_=xt[:, s * 512:(s + 1) * 512])
                        mv = lns.tile([128, 2], F32, tag="mv")
                        nc.vector.bn_aggr(out=mv, in_=stats)
                        vmax = lns.tile([128, 1], F32, tag="vmax")
                        nc.vector.tensor_scalar_max(vmax, mv[:, 1:2], eps)
                        sq = lns.tile([128, 1], F32, tag="sq")
                        nc.scalar.activation(out=sq, in_=vmax, func=AF.Sqrt)
                        scl = lns.tile([128, 1], F32, tag="scl")
                        nc.vector.reciprocal(scl, sq)
                        nshf = lns.tile([128, 1], F32, tag="nshf")
                        with nc.allow_low_precision(reason="mean*scl in f32; fine"):
                            nc.vector.tensor_scalar(
                                nshf, mv[:, 0:1], scl, -1.0, ALU.mult, ALU.mult)
                        xnt = persist.tile([128, n_sz], BF16, tag=f"xn_{b}_{dt}")
                        with nc.allow_low_precision(reason="bf16 LN apply; ~4e-3 ok"):
                            nc.vector.tensor_scalar(
                                xnt, xt, scl, nshf, ALU.mult, ALU.add)
                        xn[b, dt] = xnt

            # ---- Phase 2a: q/k projections (transposed layout) ----
            with tc.tile_pool(name="pproj", bufs=4, space="PSUM") as pproj:
                for ch in range(nch):
                    b = (ch * 512) // n_sz
                    col0 = (ch * 512) % n_sz
                    for (w_s, dst) in ((wq_s, qT), (wk_s, kT)):
                        ps = pproj.tile([IL, 512], F32, tag="ps")
                        for dt in range(nd):
                            nc.tensor.matmul(
                                ps, w_s[dt], xn[b, dt][:, col0:col0 + 512],
                                start=(dt == 0), stop=(dt == nd - 1),
                            )
                        nc.scalar.activation(out=dst[:, ch * 512:(ch + 1) * 512],
                                             in_=ps, func=AF.Copy)

            # ---- Phase 2b: v natural [tokens, inner] ----
            with tc.tile_pool(name="pv", bufs=4, space="PSUM") as pv:
                for b in range(b_sz):
                    for nj in range(njb):
                        psv = pv.tile([128, IL], F32, tag="psv")
                        for dt in range(nd):
                            nc.tensor.matmul(
                                psv, xn[b, dt][:, nj * 128:(nj + 1) * 128], wv_s[dt],
                                start=(dt == 0), stop=(dt == nd - 1),
                            )
                        t = persist.tile([128, IL], BF16, tag=f"va_{b}_{nj}")
                        nc.scalar.activation(out=t, in_=psv, func=AF.Copy)
                        va[b, nj] = t

            # ---- Phase 3: attention ----
            # stream -> psim column range: [b0h0 | b0h1 | b1h0 | b1h1]
            # (each 512 f32 = exactly one PSUM bank; b-pairs adjacent so one
            # 1024-wide exp covers both heads of a batch)
            av_u = {b: persist.tile([128, n_sz], BF16, tag=f"avu_{b}",
                                    name=f"avu_{b}")
                    for b in range(b_sz)}
            av_n = {b: persist.tile([128, n_sz], BF16, tag=f"avn_{b}",
                                    name=f"avn_{b}")
                    for b in range(b_sz)}
            zbb = {}  # (b, ni) -> [128, 512] bf16 stacked 1/Z broadcast
            with (
                tc.tile_pool(name="psim", bufs=1, space="PSUM") as psimp,
                tc.tile_pool(name="pav", bufs=1, space="PSUM") as pavp,
                tc.tile_pool(name="pz", bufs=1, space="PSUM") as pzp,
                tc.tile_pool(name="biasp", bufs=6) as biasp,
                tc.tile_pool(name="aep", bufs=1) as aep,
                tc.tile_pool(name="zc", bufs=4) as zc,
            ):
                psim = psimp.tile([128, 4 * 512], F32, tag="psim")
                ae = [aep.tile([128, 4 * 512], BF16, tag=f"ae{i}", name=f"ae{i}") for i in range(2)]
                au = [aep.tile([128, 4 * 512], BF16, tag=f"au{i}", name=f"au{i}") for i in range(2)]
                for ni in range(nic):
                    pav = {b: pavp.tile([128, 512], F32, tag=f"pav{b}",
                                        name=f"pav_{ni}_{b}") for b in range(b_sz)}
                    zt = pzp.tile([128, 512], F32, tag="zt", name=f"zt_{ni}")
                    for nj in range(njb):
                        aet, aut = ae[nj % 2], au[nj % 2]
                        bts = []
                        for h in range(HL):
                            bt = biasp.tile([128, 512], BF16, tag=f"bt{h}", name=f"bt_{ni}_{nj}_{h}")
                            nc.sync.dma_start(
                                out=bt,
                                in_=biasT[h, nj * 128:(nj + 1) * 128,
                                          ni * 512:(ni + 1) * 512],
                            )
                            bts.append(bt)
                        # sims: per batch, the two heads go to different row
                        # groups AND different PSUM banks -> concurrent
                        for b in range(b_sz):
                            for h in range(HL):
                                c0 = (b * HL + h) * 512
                                nc.tensor.matmul(
                                    psim[:, c0:c0 + 512],
                                    kT[h * DH:(h + 1) * DH,
                                       b * n_sz + nj * 128:b * n_sz + (nj + 1) * 128],
                                    qT[h * DH:(h + 1) * DH,
                                       b * n_sz + ni * 512:b * n_sz + (ni + 1) * 512],
                                    start=True, stop=True,
                                )
                            # one exp per batch-pair spanning 2 PSUM banks
                            nc.scalar.activation(
                                out=aet[:, b * 1024:(b + 1) * 1024],
                                in_=psim[:, b * 1024:(b + 1) * 1024],
                                func=AF.Exp)
                        # bias multiply (DVE 2x)
                        for b in range(b_sz):
                            for h in range(HL):
                                c0 = (b * HL + h) * 512
                                nc.vector.tensor_mul(
                                    aut[:, c0:c0 + 512], aet[:, c0:c0 + 512], bts[h])
                        # AV: col-tiled pair per batch (concurrent), stacked
                        for b in range(b_sz):
                            for h in range(HL):
                                c0 = (b * HL + h) * 512
                                nc.tensor.matmul(
                                    pav[b][h * DH:(h + 1) * DH, :],
                                    va[b, nj][:, h * DH:(h + 1) * DH],
                                    aut[:, c0:c0 + 512],
                                    start=(nj == 0), stop=(nj == njb - 1),
                                    tile_position=(0, h * DH),
                                )
                        # Z: 4 concurrent M=1 col-tiled matmuls
                        for b in range(b_sz):
                            for h in range(HL):
                                c0 = (b * HL + h) * 512
                                zrow = (b * HL + h) * 32
                                nc.tensor.matmul(
                                    zt[zrow:zrow + 1, :],
                                    ones,
                                    aut[:, c0:c0 + 512],
                                    start=(nj == 0), stop=(nj == njb - 1),
                                    tile_position=(0, zrow),
                                )
                    # ---- ni boundary: evacuate pav, reciprocal Z, roundtrip ----
                    for b in range(b_sz):
                        nc.vector.tensor_copy(
                            av_u[b][:, ni * 512:(ni + 1) * 512], pav[b])
                    zr = zc.tile([128, 512], BF16, tag="zr", name=f"zr_{ni}")
                    with nc.allow_low_precision(reason="1/Z bf16; ~4e-3 ok at 2e-2 gate"):
                        nc.vector.reciprocal(zr, zt)
                    for b in range(b_sz):
                        for h in range(HL):
                            zrow = (b * HL + h) * 32
                            nc.sync.dma_start(
                                out=zdram[b, h, 0, ni * 512:(ni + 1) * 512],
                                in_=zr[zrow:zrow + 1, :])
                    for b in range(b_sz):
                        zb = zc.tile([128, 512], BF16, tag="zbb", name=f"zbb_{ni}_{b}")
                        for h in range(HL):
                            nc.sync.dma_start(
                                out=zb[h * DH:(h + 1) * DH, :],
                                in_=zdram[b, h, :, ni * 512:(ni + 1) * 512]
                                .to_broadcast([DH, 512]))
                        zbb[b, ni] = zb

            # ---- Phase 4: normalize + output projection (stacked K=128) ----
            with (
                tc.tile_pool(name="pout", bufs=2, space="PSUM") as pout,
                tc.tile_pool(name="ost2", bufs=3) as ost2,
            ):
                for b in range(b_sz):
                    for ni in range(nic):
                        with nc.allow_low_precision(reason="bf16 attention weights"):
                            nc.vector.tensor_mul(
                                av_n[b][:, ni * 512:(ni + 1) * 512],
                                av_u[b][:, ni * 512:(ni + 1) * 512],
                                zbb[b, ni])
                for blk in range(bn // 128):
                    b = (blk * 128) // n_sz
                    r0 = (blk * 128) % n_sz
                    po = pout.tile([128, dim], F32, tag="po", name=f"po_{blk}")
                    for ci, c0 in enumerate(range(0, dim, 512)):
                        nc.tensor.matmul(
                            po[:, c0:c0 + 512],
                            av_n[b][:, r0:r0 + 128],
                            wo_full[:, c0:c0 + 512],
                            start=True, stop=True,
                        )
                    os_ = ost2.tile([128, dim], BF16, tag="os", name=f"os_{blk}")
                    # alternate evacuation between DVE and ACT
                    if blk % 2 == 0:
                        nc.vector.tensor_copy(os_[:, 0:512], po[:, 0:512])
                        nc.scalar.activation(out=os_[:, 512:1024], in_=po[:, 512:1024],
                                             func=AF.Copy)
                    else:
                        nc.scalar.activation(out=os_[:, 0:512], in_=po[:, 0:512],
                                             func=AF.Copy)
                        nc.vector.tensor_copy(os_[:, 512:1024], po[:, 512:1024])
                    nc.sync.dma_start(out=out[blk * 128:(blk + 1) * 128, :], in_=os_)
    nc.compile()
    return nc


_NC_CACHE = {}


def _get_nc(key, **kw):
    if key not in _NC_CACHE:
        _NC_CACHE[key] = build(**kw)
    return _NC_CACHE[key]


def make_in_maps(x, rel_pos_bias, g, Wq, Wkv, Wo):
    b_sz, n_sz, dim = x.shape
    inner = Wq.shape[0]
    x = np.asarray(x, np.float32)
    xTh = np.ascontiguousarray(x.transpose(0, 2, 1)).astype(BF)  # [B, DIM, N]
    gv = np.asarray(g, np.float32).reshape(1, dim)
    Wq = np.asarray(Wq, np.float32) * gv
    Wkv = np.asarray(Wkv, np.float32) * gv
    scale = DH ** -0.5
    in_maps = []
    for c in range(NCORES):
        rs, re = c * IL, (c + 1) * IL
        wq_c = np.ascontiguousarray((Wq[rs:re, :] * scale).T).astype(BF)
        wk_c = np.ascontiguousarray(Wkv[rs:re, :].T).astype(BF)
        wv_c = np.ascontiguousarray(Wkv[inner + rs:inner + re, :].T).astype(BF)
        wo_c = np.ascontiguousarray(np.asarray(Wo)[:, rs:re].T).astype(BF)
        bias_c = np.exp(np.ascontiguousarray(
            np.asarray(rel_pos_bias)[0, c * HL:(c + 1) * HL].transpose(0, 2, 1)
        )).astype(BF)
        in_maps.append({
            "xT": xTh, "wqT": wq_c, "wkT": wk_c, "wvT": wv_c,
            "woT": wo_c, "biasT": bias_c,
        })
    return in_maps


def kernel(x, rel_pos_bias, g, Wq, Wkv, Wo):
    b_sz, n_sz, dim = x.shape
    nc = _get_nc((b_sz, n_sz, dim), b_sz=b_sz, n_sz=n_sz, dim=dim)
    in_maps = make_in_maps(x, rel_pos_bias, g, Wq, Wkv, Wo)
    res = run_bass_kernel_spmd(nc, in_maps, core_ids=list(range(NCORES)))
    acc = np.zeros((b_sz * n_sz, dim), np.float32)
    for r in res.results:
        acc += np.asarray(r["out"]).astype(np.float32)
    return np.ascontiguousarray(acc.reshape(b_sz, n_sz, dim))


# revision 9
# speedup vs baseline: 1.3101x; 1.0821x over previous
"""Trainium2 Bass kernel: multi-head attention with sequence-axis layernorm
and relative position bias, sharded 8-way over heads (2 heads/core).

v2 layout strategy (per core):
  - LN over sequence axis in [d_partition, n_free] layout; stats on DVE
    (bn_stats/bn_aggr), apply on DVE via fused tensor_scalar (x*scl + nshf)
    in bf16 (4x mode); g folded into Wq/Wkv on the host.
  - qT/kT [inner_local=128, b*n] via const-weight matmuls (K=128, Nf=512).
  - v natural per (b, nj): va_full[b,nj] [128 tokens, 128 inner] bf16.
  - attention rounds (ni, nj): all 4 streams (b x h) share ONE persistent
    4-bank PSUM tile [128, 2048] f32, cols [b0h0|b0h1|b1h0|b1h1]; the two
    sims of a batch are row-tiled (K=64 at row groups 0/64) into DIFFERENT
    banks and run concurrently on the PE.
  - exp: ONE ScalarE activation per batch-pair [128, 1024] spanning 2 PSUM
    banks (amortizes the ~352-cycle ACT instruction overhead).
  - bias folded multiplicatively: host precomputes exp(biasT) bf16; DVE
    multiplies (2x mode) into au.
  - AV: col-tiled pairs — h0 -> pav[b][0:64], h1 -> pav[b][64:128] (M=64,
    col groups disjoint -> concurrent), accumulated over nj. This yields a
    head-STACKED av [128, qi] enabling a K=128 output projection.
  - Z: separate [128,512] PSUM bank; 4 concurrent M=1 col-tiled matmuls
    with ones-weights at partitions {0,32,64,96}, accumulated over nj.
  - softmax denominator: reciprocal on DVE at ni boundary, DRAM roundtrip
    to broadcast 1/Z rows across 64 partitions; normalization deferred to
    the out-projection phase (off the attention critical path).
  - out-proj: stacked K=128 matmuls (lhsT = av_n [128, tok]), PSUM->SBUF
    copies alternating DVE/ACT, bf16 partial output summed on host in f32.
"""

import numpy as np
import ml_dtypes

import concourse.bass as bass
from concourse import bacc
import concourse.mybir as mybir
import concourse.tile as tile
from concourse.bass_utils import run_bass_kernel_spmd

F32 = mybir.dt.float32
BF16 = mybir.dt.bfloat16
BF = ml_dtypes.bfloat16
AF = mybir.ActivationFunctionType
ALU = mybir.AluOpType

# full-size problem constants
B, N, DIM = 2, 2048, 1024
HEADS, DH = 16, 64
NCORES = 8
HL = HEADS // NCORES          # heads per core = 2
IL = HL * DH                  # local inner = 128
INNER = HEADS * DH            # 1024


def build(b_sz=B, n_sz=N, dim=DIM, eps=1e-5):
    """Build the per-core Bass graph (SPMD across 8 cores)."""
    nd = dim // 128               # d tiles
    nch = (b_sz * n_sz) // 512    # 512-col chunks of flattened b*n
    njb = n_sz // 128             # key tiles per batch
    nic = n_sz // 512             # query chunks per batch
    bn = b_sz * n_sz
    nsub = n_sz // 512            # bn_stats subgroups

    nc = bacc.Bacc(None, target_bir_lowering=False)
    xT = nc.declare_dram_parameter("xT", [b_sz, dim, n_sz], BF16, isOutput=False)
    wqT = nc.declare_dram_parameter("wqT", [dim, IL], BF16, isOutput=False)
    wkT = nc.declare_dram_parameter("wkT", [dim, IL], BF16, isOutput=False)
    wvT = nc.declare_dram_parameter("wvT", [dim, IL], BF16, isOutput=False)
    woT = nc.declare_dram_parameter("woT", [IL, dim], BF16, isOutput=False)
    biasT = nc.declare_dram_parameter("biasT", [HL, n_sz, n_sz], BF16, isOutput=False)  # exp(bias.T)
    out = nc.declare_dram_parameter("out", [bn, dim], BF16, isOutput=True)
    zdram = nc.dram_tensor("zscratch", [b_sz, HL, 1, n_sz], BF16)

    with tile.TileContext(nc) as tc:
        with (
            tc.tile_pool(name="consts", bufs=1) as consts,
            tc.tile_pool(name="persist", bufs=1) as persist,
        ):
            # ---- load weights ----
            wq_s, wk_s, wv_s = [], [], []
            for dt in range(nd):
                for lst, src, nm in ((wq_s, wqT, "wq"), (wk_s, wkT, "wk"), (wv_s, wvT, "wv")):
                    t = consts.tile([128, IL], BF16, tag=f"{nm}{dt}")
                    nc.sync.dma_start(out=t, in_=src[dt * 128:(dt + 1) * 128, :])
                    lst.append(t)
            wo_full = consts.tile([IL, dim], BF16, tag="wo")
            nc.sync.dma_start(out=wo_full, in_=woT[:, :])
            ones = consts.tile([128, 1], BF16, tag="ones")
            nc.vector.memset(ones, 1.0)

            xn = {}
            qT = persist.tile([IL, bn], BF16, tag="qT")
            kT = persist.tile([IL, bn], BF16, tag="kT")
            va = {}   # (b, nj) -> [128 tokens, 128 inner] bf16

            # ---- Phase 1: layernorm over sequence axis (DVE only) ----
            with (
                tc.tile_pool(name="xload", bufs=3) as xload,
                tc.tile_pool(name="lns", bufs=8) as lns,
            ):
                for b in range(b_sz):
                    for dt in range(nd):
                        xt = xload.tile([128, n_sz], BF16, tag="xt")
                        nc.sync.dma_start(out=xt, in_=xT[b, dt * 128:(dt + 1) * 128, :])
                        stats = lns.tile([128, nsub, 6], F32, tag="stats")
                        for s in range(nsub):
                            nc.vector.bn_stats(out=stats[:, s, :], in_=xt[:, s * 512:(s + 1) * 512])
                        mv = lns.tile([128, 2], F32, tag="mv")
                        nc.vector.bn_aggr(out=mv, in_=stats)
                        vmax = lns.tile([128, 1], F32, tag="vmax")
                        nc.vector.tensor_scalar_max(vmax, mv[:, 1:2], eps)
                        sq = lns.tile([128, 1], F32, tag="sq")
                        nc.scalar.activation(out=sq, in_=vmax, func=AF.Sqrt)
                        scl = lns.tile([128, 1], F32, tag="scl")
                        nc.vector.reciprocal(scl, sq)
                        nshf = lns.tile([128, 1], F32, tag="nshf")
                        with nc.allow_low_precision(reason="mean*scl in f32; fine"):
                            nc.vector.tensor_scalar(
                                nshf, mv[:, 0:1], scl, -1.0, ALU.mult, ALU.mult)
                        xnt = persist.tile([128, n_sz], BF16, tag=f"xn_{b}_{dt}")
                        with nc.allow_low_precision(reason="bf16 LN apply; ~4e-3 ok"):
                            nc.vector.tensor_scalar(
                                xnt, xt, scl, nshf, ALU.mult, ALU.add)
                        xn[b, dt] = xnt

            # ---- Phase 2a: q/k projections (transposed layout) ----
            with tc.tile_pool(name="pproj", bufs=4, space="PSUM") as pproj:
                for ch in range(nch):
                    b = (ch * 512) // n_sz
                    col0 = (ch * 512) % n_sz
                    for (w_s, dst) in ((wq_s, qT), (wk_s, kT)):
                        ps = pproj.tile([IL, 512], F32, tag="ps")
                        for dt in range(nd):
                            nc.tensor.matmul(
                                ps, w_s[dt], xn[b, dt][:, col0:col0 + 512],
                                start=(dt == 0), stop=(dt == nd - 1),
                            )
                        nc.scalar.activation(out=dst[:, ch * 512:(ch + 1) * 512],
                                             in_=ps, func=AF.Copy)

            # ---- Phase 2b: v natural [tokens, inner] ----
            with tc.tile_pool(name="pv", bufs=4, space="PSUM") as pv:
                for b in range(b_sz):
                    for nj in range(njb):
                        psv = pv.tile([128, IL], F32, tag="psv")
                        for dt in range(nd):
                            nc.tensor.matmul(
                                psv, xn[b, dt][:, nj * 128:(nj + 1) * 128], wv_s[dt],
                                start=(dt == 0), stop=(dt == nd - 1),
                            )
                        t = persist.tile([128, IL], BF16, tag=f"va_{b}_{nj}")
                        nc.scalar.activation(out=t, in_=psv, func=AF.Copy)
                        va[b, nj] = t

            # ---- Phase 3: attention ----
            # stream -> psim column range: [b0h0 | b0h1 | b1h0 | b1h1]
            # (each 512 f32 = exactly one PSUM bank; b-pairs adjacent so one
            # 1024-wide exp covers both heads of a batch)
            av_u = {b: persist.tile([128, n_sz], BF16, tag=f"avu_{b}",
                                    name=f"avu_{b}")
                    for b in range(b_sz)}
            av_n = {b: persist.tile([128, n_sz], BF16, tag=f"avn_{b}",
                                    name=f"avn_{b}")
                    for b in range(b_sz)}
            zbb = {}  # (b, ni) -> [128, 512] bf16 stacked 1/Z broadcast
            with (
                tc.tile_pool(name="psim", bufs=1, space="PSUM") as psimp,
                tc.tile_pool(name="pav", bufs=1, space="PSUM") as pavp,
                tc.tile_pool(name="pz", bufs=1, space="PSUM") as pzp,
                tc.tile_pool(name="biasp", bufs=6) as biasp,
                tc.tile_pool(name="aep", bufs=1) as aep,
                tc.tile_pool(name="zc", bufs=4) as zc,
            ):
                psim = psimp.tile([128, 4 * 512], F32, tag="psim")
                ae = [aep.tile([128, 4 * 512], BF16, tag=f"ae{i}", name=f"ae{i}") for i in range(2)]
                au = [aep.tile([128, 4 * 512], BF16, tag=f"au{i}", name=f"au{i}") for i in range(2)]
                def av_z(ni, nj, aut, pav, zt):
                    # AV: col-tiled pair per batch (concurrent), stacked
                    for b in range(b_sz):
                        for h in range(HL):
                            c0 = (b * HL + h) * 512
                            nc.tensor.matmul(
                                pav[b][h * DH:(h + 1) * DH, :],
                                va[b, nj][:, h * DH:(h + 1) * DH],
                                aut[:, c0:c0 + 512],
                                start=(nj == 0), stop=(nj == njb - 1),
                                tile_position=(0, h * DH),
                            )
                    # Z: 4 concurrent M=1 col-tiled matmuls
                    for b in range(b_sz):
                        for h in range(HL):
                            c0 = (b * HL + h) * 512
                            zrow = (b * HL + h) * 32
                            nc.tensor.matmul(
                                zt[zrow:zrow + 1, :],
                                ones,
                                aut[:, c0:c0 + 512],
                                start=(nj == 0), stop=(nj == njb - 1),
                                tile_position=(0, zrow),
                            )

                for ni in range(nic):
                    pav = {b: pavp.tile([128, 512], F32, tag=f"pav{b}",
                                        name=f"pav_{ni}_{b}") for b in range(b_sz)}
                    zt = pzp.tile([128, 512], F32, tag="zt", name=f"zt_{ni}")
                    for nj in range(njb):
                        aet, aut = ae[nj % 2], au[nj % 2]
                        bts = []
                        for h in range(HL):
                            bt = biasp.tile([128, 512], BF16, tag=f"bt{h}", name=f"bt_{ni}_{nj}_{h}")
                            nc.sync.dma_start(
                                out=bt,
                                in_=biasT[h, nj * 128:(nj + 1) * 128,
                                          ni * 512:(ni + 1) * 512],
                            )
                            bts.append(bt)
                        # sims: per batch, the two heads go to different row
                        # groups AND different PSUM banks -> concurrent
                        for b in range(b_sz):
                            for h in range(HL):
                                c0 = (b * HL + h) * 512
                                nc.tensor.matmul(
                                    psim[:, c0:c0 + 512],
                                    kT[h * DH:(h + 1) * DH,
                                       b * n_sz + nj * 128:b * n_sz + (nj + 1) * 128],
                                    qT[h * DH:(h + 1) * DH,
                                       b * n_sz + ni * 512:b * n_sz + (ni + 1) * 512],
                                    start=True, stop=True,
                                )
                            # one exp per batch-pair spanning 2 PSUM banks
                            nc.scalar.activation(
                                out=aet[:, b * 1024:(b + 1) * 1024],
                                in_=psim[:, b * 1024:(b + 1) * 1024],
                                func=AF.Exp)
                        # bias multiply (DVE 2x)
                        for b in range(b_sz):
                            for h in range(HL):
                                c0 = (b * HL + h) * 512
                                nc.vector.tensor_mul(
                                    aut[:, c0:c0 + 512], aet[:, c0:c0 + 512], bts[h])
                        # AV/Z for the PREVIOUS round: software pipeline so the
                        # PE never sits behind this round's exp->mult latency
                        if nj > 0:
                            av_z(ni, nj - 1, au[(nj - 1) % 2], pav, zt)
                    av_z(ni, njb - 1, au[(njb - 1) % 2], pav, zt)
                    # ---- ni boundary: evacuate pav, reciprocal Z, roundtrip ----
                    for b in range(b_sz):
                        nc.vector.tensor_copy(
                            av_u[b][:, ni * 512:(ni + 1) * 512], pav[b])
                    zrf = zc.tile([128, 512], F32, tag="zrf", name=f"zrf_{ni}")
                    nc.vector.reciprocal_approx_fast(zrf, zt)
                    zr = zc.tile([128, 512], BF16, tag="zr", name=f"zr_{ni}")
                    with nc.allow_low_precision(reason="1/Z bf16; ~4e-3 ok at 2e-2 gate"):
                        nc.vector.tensor_copy(zr, zrf)
                    for b in range(b_sz):
                        for h in range(HL):
                            zrow = (b * HL + h) * 32
                            nc.sync.dma_start(
                                out=zdram[b, h, 0, ni * 512:(ni + 1) * 512],
                                in_=zr[zrow:zrow + 1, :])
                    for b in range(b_sz):
                        zb = zc.tile([128, 512], BF16, tag="zbb", name=f"zbb_{ni}_{b}")
                        for h in range(HL):
                            nc.sync.dma_start(
                                out=zb[h * DH:(h + 1) * DH, :],
                                in_=zdram[b, h, :, ni * 512:(ni + 1) * 512]
                                .to_broadcast([DH, 512]))
                        zbb[b, ni] = zb
                        with nc.allow_low_precision(reason="bf16 attention weights"):
                            nc.vector.tensor_mul(
                                av_n[b][:, ni * 512:(ni + 1) * 512],
                                av_u[b][:, ni * 512:(ni + 1) * 512],
                                zb)

            # ---- Phase 4: normalize + output projection (stacked K=128) ----
            with (
                tc.tile_pool(name="pout", bufs=4, space="PSUM") as pout,
                tc.tile_pool(name="ost2", bufs=4) as ost2,
            ):
                for blk in range(bn // 128):
                    b = (blk * 128) // n_sz
                    r0 = (blk * 128) % n_sz
                    po = pout.tile([128, dim], F32, tag="po", name=f"po_{blk}")
                    for ci, c0 in enumerate(range(0, dim, 512)):
                        nc.tensor.matmul(
                            po[:, c0:c0 + 512],
                            av_n[b][:, r0:r0 + 128],
                            wo_full[:, c0:c0 + 512],
                            start=True, stop=True,
                        )
                    os_ = ost2.tile([128, dim], BF16, tag="os", name=f"os_{blk}")
                    # alternate evacuation between DVE and ACT
                    if blk % 2 == 0:
                        nc.vector.tensor_copy(os_[:, 0:512], po[:, 0:512])
                        nc.scalar.activation(out=os_[:, 512:1024], in_=po[:, 512:1024],
                                             func=AF.Copy)
                    else:
                        nc.scalar.activation(out=os_[:, 0:512], in_=po[:, 0:512],
                                             func=AF.Copy)
                        nc.vector.tensor_copy(os_[:, 512:1024], po[:, 512:1024])
                    nc.sync.dma_start(out=out[blk * 128:(blk + 1) * 128, :], in_=os_)
    nc.compile()
    return nc


_NC_CACHE = {}


def _get_nc(key, **kw):
    if key not in _NC_CACHE:
        _NC_CACHE[key] = build(**kw)
    return _NC_CACHE[key]


def make_in_maps(x, rel_pos_bias, g, Wq, Wkv, Wo):
    b_sz, n_sz, dim = x.shape
    inner = Wq.shape[0]
    x = np.asarray(x, np.float32)
    xTh = np.ascontiguousarray(x.transpose(0, 2, 1)).astype(BF)  # [B, DIM, N]
    gv = np.asarray(g, np.float32).reshape(1, dim)
    Wq = np.asarray(Wq, np.float32) * gv
    Wkv = np.asarray(Wkv, np.float32) * gv
    scale = DH ** -0.5
    in_maps = []
    for c in range(NCORES):
        rs, re = c * IL, (c + 1) * IL
        wq_c = np.ascontiguousarray((Wq[rs:re, :] * scale).T).astype(BF)
        wk_c = np.ascontiguousarray(Wkv[rs:re, :].T).astype(BF)
        wv_c = np.ascontiguousarray(Wkv[inner + rs:inner + re, :].T).astype(BF)
        wo_c = np.ascontiguousarray(np.asarray(Wo)[:, rs:re].T).astype(BF)
        bias_c = np.exp(np.ascontiguousarray(
            np.asarray(rel_pos_bias)[0, c * HL:(c + 1) * HL].transpose(0, 2, 1)
        )).astype(BF)
        in_maps.append({
            "xT": xTh, "wqT": wq_c, "wkT": wk_c, "wvT": wv_c,
            "woT": wo_c, "biasT": bias_c,
        })
    return in_maps


def kernel(x, rel_pos_bias, g, Wq, Wkv, Wo):
    b_sz, n_sz, dim = x.shape
    nc = _get_nc((b_sz, n_sz, dim), b_sz=b_sz, n_sz=n_sz, dim=dim)
    in_maps = make_in_maps(x, rel_pos_bias, g, Wq, Wkv, Wo)
    res = run_bass_kernel_spmd(nc, in_maps, core_ids=list(range(NCORES)))
    acc = np.zeros((b_sz * n_sz, dim), np.float32)
    for r in res.results:
        acc += np.asarray(r["out"]).astype(np.float32)
    return np.ascontiguousarray(acc.reshape(b_sz, n_sz, dim))


# revision 13
# speedup vs baseline: 1.3513x; 1.0314x over previous
"""Trainium2 Bass kernel: multi-head attention with sequence-axis layernorm
and relative position bias, sharded 8-way over heads (2 heads/core).

v2 layout strategy (per core):
  - LN over sequence axis in [d_partition, n_free] layout; stats on DVE
    (bn_stats/bn_aggr), apply on DVE via fused tensor_scalar (x*scl + nshf)
    in bf16 (4x mode); g folded into Wq/Wkv on the host.
  - qT/kT [inner_local=128, b*n] via const-weight matmuls (K=128, Nf=512).
  - v natural per (b, nj): va_full[b,nj] [128 tokens, 128 inner] bf16.
  - attention rounds (ni, nj): all 4 streams (b x h) share ONE persistent
    4-bank PSUM tile [128, 2048] f32, cols [b0h0|b0h1|b1h0|b1h1]; the two
    sims of a batch are row-tiled (K=64 at row groups 0/64) into DIFFERENT
    banks and run concurrently on the PE.
  - exp: ONE ScalarE activation per batch-pair [128, 1024] spanning 2 PSUM
    banks (amortizes the ~352-cycle ACT instruction overhead).
  - bias folded multiplicatively: host precomputes exp(biasT) bf16; DVE
    multiplies (2x mode) into au.
  - AV: col-tiled pairs — h0 -> pav[b][0:64], h1 -> pav[b][64:128] (M=64,
    col groups disjoint -> concurrent), accumulated over nj. This yields a
    head-STACKED av [128, qi] enabling a K=128 output projection.
  - Z: separate [128,512] PSUM bank; 4 concurrent M=1 col-tiled matmuls
    with ones-weights at partitions {0,32,64,96}, accumulated over nj.
  - softmax denominator: reciprocal on DVE at ni boundary, DRAM roundtrip
    to broadcast 1/Z rows across 64 partitions; normalization deferred to
    the out-projection phase (off the attention critical path).
  - out-proj: stacked K=128 matmuls (lhsT = av_n [128, tok]), PSUM->SBUF
    copies alternating DVE/ACT, bf16 partial output summed on host in f32.
"""

import numpy as np
import ml_dtypes

import concourse.bass as bass
from concourse import bacc
import concourse.mybir as mybir
import concourse.tile as tile
from concourse.bass_utils import run_bass_kernel_spmd

F32 = mybir.dt.float32
BF16 = mybir.dt.bfloat16
BF = ml_dtypes.bfloat16
AF = mybir.ActivationFunctionType
ALU = mybir.AluOpType

# full-size problem constants
B, N, DIM = 2, 2048, 1024
HEADS, DH = 16, 64
NCORES = 8
HL = HEADS // NCORES          # heads per core = 2
IL = HL * DH                  # local inner = 128
INNER = HEADS * DH            # 1024


def build(b_sz=B, n_sz=N, dim=DIM, eps=1e-5):
    """Build the per-core Bass graph (SPMD across 8 cores)."""
    nd = dim // 128               # d tiles
    nch = (b_sz * n_sz) // 512    # 512-col chunks of flattened b*n
    njb = n_sz // 128             # key tiles per batch
    nic = n_sz // 512             # query chunks per batch
    bn = b_sz * n_sz
    nsub = n_sz // 512            # bn_stats subgroups

    nc = bacc.Bacc(None, target_bir_lowering=False)
    xT = nc.declare_dram_parameter("xT", [b_sz, dim, n_sz], BF16, isOutput=False)
    wqT = nc.declare_dram_parameter("wqT", [dim, IL], BF16, isOutput=False)
    wkT = nc.declare_dram_parameter("wkT", [dim, IL], BF16, isOutput=False)
    wvT = nc.declare_dram_parameter("wvT", [dim, IL], BF16, isOutput=False)
    woT = nc.declare_dram_parameter("woT", [IL, dim], BF16, isOutput=False)
    biasT = nc.declare_dram_parameter("biasT", [HL, n_sz, n_sz], BF16, isOutput=False)  # exp(bias.T)
    out = nc.declare_dram_parameter("out", [bn, dim], BF16, isOutput=True)
    zdram = nc.dram_tensor("zscratch", [b_sz, HL, 1, n_sz], BF16)

    with tile.TileContext(nc) as tc:
        with (
            tc.tile_pool(name="consts", bufs=1) as consts,
            tc.tile_pool(name="persist", bufs=1) as persist,
        ):
            # ---- load weights ----
            wq_s, wk_s, wv_s = [], [], []
            for dt in range(nd):
                for lst, src, nm in ((wq_s, wqT, "wq"), (wk_s, wkT, "wk"), (wv_s, wvT, "wv")):
                    t = consts.tile([128, IL], BF16, tag=f"{nm}{dt}")
                    nc.sync.dma_start(out=t, in_=src[dt * 128:(dt + 1) * 128, :])
                    lst.append(t)
            wo_full = consts.tile([IL, dim], BF16, tag="wo")
            nc.sync.dma_start(out=wo_full, in_=woT[:, :])
            ones = consts.tile([128, 1], BF16, tag="ones")
            nc.vector.memset(ones, 1.0)

            xn = {}
            qT = persist.tile([IL, bn], BF16, tag="qT")
            kT = persist.tile([IL, bn], BF16, tag="kT")
            va = {}   # (b, nj) -> [128 tokens, 128 inner] bf16

            # ---- Phase 1: layernorm over sequence axis ----
            # mean/var via DVE bn_stats for half the tiles, via ScalarE
            # Square/Identity+accum_out for the other half (the two engines
            # run the stats concurrently; DVE was the pre-phase bottleneck)
            inv_n = 1.0 / n_sz
            with (
                tc.tile_pool(name="xload", bufs=4) as xload,
                tc.tile_pool(name="lns", bufs=8) as lns,
                tc.tile_pool(name="lnscr", bufs=2) as lnscr,
            ):
                for b in range(b_sz):
                    for dt in range(nd):
                        xt = xload.tile([128, n_sz], BF16, tag="xt",
                                        name=f"xt_{b}_{dt}")
                        nc.sync.dma_start(out=xt, in_=xT[b, dt * 128:(dt + 1) * 128, :])
                        mv = lns.tile([128, 2], F32, tag="mv", name=f"mv_{b}_{dt}")
                        if (b * nd + dt) % 2 == 0:
                            stats = lns.tile([128, nsub, 6], F32, tag="stats",
                                             name=f"st_{b}_{dt}")
                            for s in range(nsub):
                                nc.vector.bn_stats(out=stats[:, s, :],
                                                   in_=xt[:, s * 512:(s + 1) * 512])
                            nc.vector.bn_aggr(out=mv, in_=stats)
                        else:
                            scr = lnscr.tile([128, n_sz], BF16, tag="scr",
                                             name=f"scr_{b}_{dt}")
                            sums = lns.tile([128, 2], F32, tag="sums",
                                            name=f"sums_{b}_{dt}")
                            nc.scalar.activation(out=scr, in_=xt, func=AF.Identity,
                                                 accum_out=sums[:, 0:1])
                            nc.scalar.activation(out=scr, in_=xt, func=AF.Square,
                                                 accum_out=sums[:, 1:2])
                            # mean = sum/n ; var = sumsq/n - mean^2
                            nc.vector.tensor_scalar_mul(mv[:, 0:1], sums[:, 0:1], inv_n)
                            msq = lns.tile([128, 1], F32, tag="msq",
                                           name=f"msq_{b}_{dt}")
                            nc.vector.tensor_mul(msq, mv[:, 0:1], mv[:, 0:1])
                            with nc.allow_low_precision(reason="var f32"):
                                nc.vector.tensor_scalar(
                                    mv[:, 1:2], sums[:, 1:2], inv_n, msq,
                                    ALU.mult, ALU.subtract)
                        vmax = lns.tile([128, 1], F32, tag="vmax", name=f"vm_{b}_{dt}")
                        nc.vector.tensor_scalar_max(vmax, mv[:, 1:2], eps)
                        sq = lns.tile([128, 1], F32, tag="sq", name=f"sq_{b}_{dt}")
                        nc.scalar.activation(out=sq, in_=vmax, func=AF.Sqrt)
                        scl = lns.tile([128, 1], F32, tag="scl", name=f"scl_{b}_{dt}")
                        nc.vector.reciprocal(scl, sq)
                        nshf = lns.tile([128, 1], F32, tag="nshf", name=f"ns_{b}_{dt}")
                        with nc.allow_low_precision(reason="mean*scl in f32; fine"):
                            nc.vector.tensor_scalar(
                                nshf, mv[:, 0:1], scl, -1.0, ALU.mult, ALU.mult)
                        xnt = persist.tile([128, n_sz], BF16, tag=f"xn_{b}_{dt}")
                        with nc.allow_low_precision(reason="bf16 LN apply; ~4e-3 ok"):
                            nc.vector.tensor_scalar(
                                xnt, xt, scl, nshf, ALU.mult, ALU.add)
                        xn[b, dt] = xnt

            # ---- Phase 2a: q/k projections (transposed layout) ----
            with tc.tile_pool(name="pproj", bufs=4, space="PSUM") as pproj:
                for ch in range(nch):
                    b = (ch * 512) // n_sz
                    col0 = (ch * 512) % n_sz
                    for (w_s, dst) in ((wq_s, qT), (wk_s, kT)):
                        ps = pproj.tile([IL, 512], F32, tag="ps")
                        for dt in range(nd):
                            nc.tensor.matmul(
                                ps, w_s[dt], xn[b, dt][:, col0:col0 + 512],
                                start=(dt == 0), stop=(dt == nd - 1),
                            )
                        nc.scalar.activation(out=dst[:, ch * 512:(ch + 1) * 512],
                                             in_=ps, func=AF.Copy)

            # ---- Phase 2b: v natural [tokens, inner] ----
            with tc.tile_pool(name="pv", bufs=4, space="PSUM") as pv:
                for b in range(b_sz):
                    for nj in range(njb):
                        psv = pv.tile([128, IL], F32, tag="psv")
                        for dt in range(nd):
                            nc.tensor.matmul(
                                psv, xn[b, dt][:, nj * 128:(nj + 1) * 128], wv_s[dt],
                                start=(dt == 0), stop=(dt == nd - 1),
                            )
                        t = persist.tile([128, IL], BF16, tag=f"va_{b}_{nj}")
                        nc.scalar.activation(out=t, in_=psv, func=AF.Copy)
                        va[b, nj] = t

            # ---- Phase 3: attention ----
            # stream -> psim column range: [b0h0 | b0h1 | b1h0 | b1h1]
            # (each 512 f32 = exactly one PSUM bank; b-pairs adjacent so one
            # 1024-wide exp covers both heads of a batch)
            av_u = {b: persist.tile([128, n_sz], BF16, tag=f"avu_{b}",
                                    name=f"avu_{b}")
                    for b in range(b_sz)}
            av_n = {b: persist.tile([128, n_sz], BF16, tag=f"avn_{b}",
                                    name=f"avn_{b}")
                    for b in range(b_sz)}
            zbb = {}  # (b, ni) -> [128, 512] bf16 stacked 1/Z broadcast
            with (
                tc.tile_pool(name="psim", bufs=1, space="PSUM") as psimp,
                tc.tile_pool(name="pav", bufs=1, space="PSUM") as pavp,
                tc.tile_pool(name="pz", bufs=1, space="PSUM") as pzp,
                tc.tile_pool(name="pop", bufs=1, space="PSUM") as popp,
                tc.tile_pool(name="biasp", bufs=6) as biasp,
                tc.tile_pool(name="aep", bufs=1) as aep,
                tc.tile_pool(name="osp", bufs=4) as osp,
                tc.tile_pool(name="zc", bufs=4) as zc,
            ):
                psim = psimp.tile([128, 4, 512], F32, tag="psim")
                ae = [aep.tile([128, 4, 512], BF16, tag=f"ae{i}", name=f"ae{i}")
                      for i in range(2)]
                au = [aep.tile([128, 4, 512], BF16, tag=f"au{i}", name=f"au{i}")
                      for i in range(2)]

                def av_z(nj, aut, pav, zt):
                    # AV: col-tiled pair per batch (concurrent), stacked;
                    # Z: 4 concurrent M=1 col-tiled matmuls
                    for b in range(b_sz):
                        for h in range(HL):
                            s = b * HL + h
                            nc.tensor.matmul(
                                pav[b][h * DH:(h + 1) * DH, :],
                                va[b, nj][:, h * DH:(h + 1) * DH],
                                aut[:, s, :],
                                start=(nj == 0), stop=(nj == njb - 1),
                                tile_position=(0, h * DH),
                            )
                    for b in range(b_sz):
                        for h in range(HL):
                            s = b * HL + h
                            nc.tensor.matmul(
                                zt[s * 32:s * 32 + 1, :],
                                ones,
                                aut[:, s, :],
                                start=(nj == 0), stop=(nj == njb - 1),
                                tile_position=(0, s * 32),
                            )

                def po_chunk(ni, c):
                    # output projection for one 512-dim half of a 128-token
                    # block of query chunk ni (interleaved into later rounds)
                    tb, half = c // 2, c % 2
                    b, r = tb // (512 // 128), tb % (512 // 128)
                    r0 = ni * 512 + r * 128
                    po = popp.tile([128, 512], F32, tag="po", name=f"po_{ni}_{c}")
                    nc.tensor.matmul(
                        po, av_n[b][:, r0:r0 + 128],
                        wo_full[:, half * 512:(half + 1) * 512],
                        start=True, stop=True,
                    )
                    os_ = osp.tile([128, 512], BF16, tag="os", name=f"os_{ni}_{c}")
                    nc.vector.tensor_copy(os_, po)
                    nc.sync.dma_start(
                        out=out[b * n_sz + r0:b * n_sz + r0 + 128,
                                half * 512:(half + 1) * 512],
                        in_=os_)

                for ni in range(nic):
                    pav = {b: pavp.tile([128, 512], F32, tag=f"pav{b}",
                                        name=f"pav_{ni}_{b}") for b in range(b_sz)}
                    zt = pzp.tile([128, 512], F32, tag="zt", name=f"zt_{ni}")
                    for nj in range(njb):
                        aet, aut = ae[nj % 2], au[nj % 2]
                        # both heads' bias block in ONE DMA: [2,128,512] ->
                        # [128, 2, 512] (= [h0 | h1] along free axis)
                        bt2 = biasp.tile([128, HL, 512], BF16, tag="bt2",
                                         name=f"bt_{ni}_{nj}")
                        nc.sync.dma_start(
                            out=bt2,
                            in_=biasT[:, nj * 128:(nj + 1) * 128,
                                      ni * 512:(ni + 1) * 512]
                            .rearrange("h p q -> p h q"),
                        )
                        # sims: per batch, the two heads go to different row
                        # groups AND different PSUM banks -> concurrent
                        for b in range(b_sz):
                            for h in range(HL):
                                s = b * HL + h
                                nc.tensor.matmul(
                                    psim[:, s, :],
                                    kT[h * DH:(h + 1) * DH,
                                       b * n_sz + nj * 128:b * n_sz + (nj + 1) * 128],
                                    qT[h * DH:(h + 1) * DH,
                                       b * n_sz + ni * 512:b * n_sz + (ni + 1) * 512],
                                    start=True, stop=True,
                                )
                            # one exp per batch-pair spanning 2 PSUM banks
                            nc.scalar.activation(
                                out=aet[:, b * HL:(b + 1) * HL, :],
                                in_=psim[:, b * HL:(b + 1) * HL, :],
                                func=AF.Exp)
                        # bias multiply: ONE DVE op per batch-half, so all four
                        # AV/Z matmuls of a half become ready together and the
                        # scheduler packs them into concurrent col/row-groups
                        for b in range(b_sz):
                            nc.vector.tensor_mul(
                                aut[:, b * HL:(b + 1) * HL, :],
                                aet[:, b * HL:(b + 1) * HL, :],
                                bt2)
                        # AV/Z for the PREVIOUS round: software pipeline so the
                        # PE never sits behind this round's exp->mult latency
                        if nj > 0:
                            av_z(nj - 1, au[(nj - 1) % 2], pav, zt)
                        # interleave the previous query-chunk's output proj
                        if ni > 0:
                            po_chunk(ni - 1, nj)
                    av_z(njb - 1, au[(njb - 1) % 2], pav, zt)
                    # ---- ni boundary: evacuate pav, reciprocal Z, roundtrip ----
                    for b in range(b_sz):
                        nc.vector.tensor_copy(
                            av_u[b][:, ni * 512:(ni + 1) * 512], pav[b])
                    zrf = zc.tile([128, 512], F32, tag="zrf", name=f"zrf_{ni}")
                    nc.vector.reciprocal_approx_fast(zrf, zt)
                    zr = zc.tile([128, 512], BF16, tag="zr", name=f"zr_{ni}")
                    with nc.allow_low_precision(reason="1/Z bf16; ~4e-3 ok at 2e-2 gate"):
                        nc.vector.tensor_copy(zr, zrf)
                    for b in range(b_sz):
                        for h in range(HL):
                            s = b * HL + h
                            nc.sync.dma_start(
                                out=zdram[b, h, 0, ni * 512:(ni + 1) * 512],
                                in_=zr[s * 32:s * 32 + 1, :])
                    for b in range(b_sz):
                        zb = zc.tile([128, 512], BF16, tag="zbb", name=f"zbb_{ni}_{b}")
                        for h in range(HL):
                            nc.sync.dma_start(
                                out=zb[h * DH:(h + 1) * DH, :],
                                in_=zdram[b, h, :, ni * 512:(ni + 1) * 512]
                                .to_broadcast([DH, 512]))
                        with nc.allow_low_precision(reason="bf16 attention weights"):
                            nc.vector.tensor_mul(
                                av_n[b][:, ni * 512:(ni + 1) * 512],
                                av_u[b][:, ni * 512:(ni + 1) * 512],
                                zb)
                # last query-chunk's output projection (epilogue)
                for c in range(2 * (512 // 128) * b_sz):
                    po_chunk(nic - 1, c)
    nc.compile()
    return nc


_NC_CACHE = {}


def _get_nc(key, **kw):
    if key not in _NC_CACHE:
        _NC_CACHE[key] = build(**kw)
    return _NC_CACHE[key]


def make_in_maps(x, rel_pos_bias, g, Wq, Wkv, Wo):
    b_sz, n_sz, dim = x.shape
    inner = Wq.shape[0]
    x = np.asarray(x, np.float32)
    xTh = np.ascontiguousarray(x.transpose(0, 2, 1)).astype(BF)  # [B, DIM, N]
    gv = np.asarray(g, np.float32).reshape(1, dim)
    Wq = np.asarray(Wq, np.float32) * gv
    Wkv = np.asarray(Wkv, np.float32) * gv
    scale = DH ** -0.5
    in_maps = []
    for c in range(NCORES):
        rs, re = c * IL, (c + 1) * IL
        wq_c = np.ascontiguousarray((Wq[rs:re, :] * scale).T).astype(BF)
        wk_c = np.ascontiguousarray(Wkv[rs:re, :].T).astype(BF)
        wv_c = np.ascontiguousarray(Wkv[inner + rs:inner + re, :].T).astype(BF)
        wo_c = np.ascontiguousarray(np.asarray(Wo)[:, rs:re].T).astype(BF)
        bias_c = np.exp(np.ascontiguousarray(
            np.asarray(rel_pos_bias)[0, c * HL:(c + 1) * HL].transpose(0, 2, 1)
        )).astype(BF)
        in_maps.append({
            "xT": xTh, "wqT": wq_c, "wkT": wk_c, "wvT": wv_c,
            "woT": wo_c, "biasT": bias_c,
        })
    return in_maps


def kernel(x, rel_pos_bias, g, Wq, Wkv, Wo):
    b_sz, n_sz, dim = x.shape
    nc = _get_nc((b_sz, n_sz, dim), b_sz=b_sz, n_sz=n_sz, dim=dim)
    in_maps = make_in_maps(x, rel_pos_bias, g, Wq, Wkv, Wo)
    res = run_bass_kernel_spmd(nc, in_maps, core_ids=list(range(NCORES)))
    acc = np.zeros((b_sz * n_sz, dim), np.float32)
    for r in res.results:
        acc += np.asarray(r["out"]).astype(np.float32)
    return np.ascontiguousarray(acc.reshape(b_sz, n_sz, dim))
